# revision 10
# baseline (speedup 1.0000x reference)
"""Chamfer loss kernel for 8 Trainium2 NeuronCores.

Math: dist2[n, m] = ||pred_n||^2 + ||label_m||^2 - 2 pred_n . label_m
computed as a single K=16 matmul with augmented operands. Every operand
is split into an fp16 (hi, lo) pair (Dekker-style), so the fp16 matmul
(1 cycle/row on PE, vs 4 for fp32) reproduces fp32-level accuracy:
    cross terms: (ah+al).(ch+cl) -> 12 rows of pairwise products
    norm terms:  ||p||^2 and ||l||^2 as hi/lo pairs against ones -> 4 rows
Sharding: pred rows split across the 8 cores (1024 each); labels replicated.
Each core emits:
    rowmin [128, 8]  - min_m dist2 for its 1024 preds (partition p, block a)
    colmin [1, 8192] - min over its local preds for every label
Host: sqrt+mean of rowmins; cross-core min of colmins then sqrt+mean.
sqrt is monotonic so mins are taken on squared distances.
"""

import sys

for _p in ("/opt/trn_rl_repo", "/root/.axon_site/_ro/trn_rl_repo"):
    if _p not in sys.path:
        sys.path.append(_p)

import numpy as np

import concourse.bacc as bacc
import concourse.bass as bass
import concourse.mybir as mybir
import concourse.tile as tile
from concourse import bass_isa
from concourse.bass_utils import run_bass_kernel_spmd

F32 = mybir.dt.float32
F16 = mybir.dt.float16
KAUG = 16  # augmented contraction dim (fp16 hi/lo pairs)

N_CORES = 8
N = 8192  # preds (total)
M = 8192  # labels
NLOC = N // N_CORES  # preds per core
P = 128  # partitions
NBLK = NLOC // P  # pred blocks per core (8)
SUPER = 2048  # psum supertile width (4 banks)
MSUP = M // SUPER  # label supertiles (4)
MM = 512  # moving width per matmul

_nc_cache = None


def _build_nc():
    nc = bacc.Bacc(None, target_bir_lowering=False)

    predT_d = nc.dram_tensor("predT", [KAUG, NLOC], F16, kind="ExternalInput")
    labelT_d = nc.dram_tensor("labelT", [KAUG, M], F16, kind="ExternalInput")
    rowmin_d = nc.dram_tensor("rowmin", [P, NBLK], F32, kind="ExternalOutput")
    colmin_d = nc.dram_tensor("colmin", [1, M], F32, kind="ExternalOutput")

    AX = mybir.AxisListType
    OP = mybir.AluOpType

    with tile.TileContext(nc) as tc:
        with (
            tc.tile_pool(name="const", bufs=1) as cpool,
            tc.tile_pool(name="psum", bufs=2, space=bass.MemorySpace.PSUM) as ppool,
            tc.tile_pool(name="work", bufs=4) as wpool,
        ):
            predT_s = cpool.tile([KAUG, NLOC], F16)
            labelT_s = cpool.tile([KAUG, M], F16)
            nc.sync.dma_start(predT_s[:], predT_d[:])
            nc.sync.dma_start(labelT_s[:], labelT_d[:])

            colacc = cpool.tile([P, M], F32)
            rowmin_s = cpool.tile([P, NBLK], F32)

            for a in range(NBLK):
                row4 = wpool.tile([P, MSUP], F32, tag="row4")
                for b in range(MSUP):
                    ps = ppool.tile([P, SUPER], F32, tag="ps")
                    for k in range(SUPER // MM):
                        off = b * SUPER + k * MM
                        nc.tensor.matmul(
                            ps[:, k * MM : (k + 1) * MM],
                            predT_s[:, a * P : (a + 1) * P],
                            labelT_s[:, off : off + MM],
                            start=True,
                            stop=True,
                        )
                    # rowmin partial for this supertile
                    nc.vector.tensor_reduce(
                        row4[:, b : b + 1], ps[:], axis=AX.X, op=OP.min
                    )
                    # colmin accumulate, negated (cross-lane reduce only has max)
                    dst = colacc[:, b * SUPER : (b + 1) * SUPER]
                    if a == 0:
                        nc.scalar.mul(dst, ps[:], -1.0)
                    else:
                        nc.vector.scalar_tensor_tensor(
                            dst, ps[:], -1.0, dst, op0=OP.mult, op1=OP.max
                        )
                nc.vector.tensor_reduce(
                    rowmin_s[:, a : a + 1], row4[:], axis=AX.X, op=OP.min
                )

            # fold 128 partitions for the label-side mins: max of negated
            colred = cpool.tile([P, M], F32)
            nc.gpsimd.partition_all_reduce(
                colred[:], colacc[:], channels=P, reduce_op=bass_isa.ReduceOp.max
            )

            nc.sync.dma_start(rowmin_d[:], rowmin_s[:])
            nc.sync.dma_start(colmin_d[:], colred[0:1, :])

    nc.finalize()
    return nc


def _get_nc():
    global _nc_cache
    if _nc_cache is None:
        _nc_cache = _build_nc()
    return _nc_cache


def _make_inputs(pred, label):
    f16 = np.float16
    m2p = -2.0 * pred  # exact in fp32
    ah = m2p.astype(f16)
    al = (m2p - ah.astype(np.float32)).astype(f16)
    ch = label.astype(f16)
    cl = (label - ch.astype(np.float32)).astype(f16)
    pn = (pred.astype(np.float64) ** 2).sum(axis=1)
    ln = (label.astype(np.float64) ** 2).sum(axis=1)
    pnh = pn.astype(f16)
    pnl = (pn - pnh.astype(np.float64)).astype(f16)
    lnh = ln.astype(f16)
    lnl = (ln - lnh.astype(np.float64)).astype(f16)

    predT = np.empty((KAUG, N), f16)
    labelT = np.empty((KAUG, M), f16)
    predT[0:3] = ah.T
    predT[3:6] = ah.T
    predT[6:9] = al.T
    predT[9:12] = al.T
    predT[12] = pnh
    predT[13] = pnl
    predT[14] = 1.0
    predT[15] = 1.0
    labelT[0:3] = ch.T
    labelT[3:6] = cl.T
    labelT[6:9] = ch.T
    labelT[9:12] = cl.T
    labelT[12] = 1.0
    labelT[13] = 1.0
    labelT[14] = lnh
    labelT[15] = lnl
    return [
        {
            "predT": np.ascontiguousarray(predT[:, c * NLOC : (c + 1) * NLOC]),
            "labelT": labelT,
        }
        for c in range(N_CORES)
    ]


def _finish(results):
    rowmins = np.stack([r["rowmin"] for r in results])  # [8, 128, 8]
    # colmin output is negated (device folds with max); undo here
    colmins = -np.stack([r["colmin"][0] for r in results])  # [8, 8192]
    dis_xy = np.sqrt(np.maximum(rowmins, 0.0)).mean(dtype=np.float64)
    colmin = colmins.min(axis=0)
    dis_yx = np.sqrt(np.maximum(colmin, 0.0)).mean(dtype=np.float64)
    return np.float32(dis_xy + dis_yx)


def _run(pred, label, trace=False, **kw):
    nc = _get_nc()
    in_maps = _make_inputs(pred, label)
    res = run_bass_kernel_spmd(nc, in_maps, list(range(N_CORES)), trace=trace, **kw)
    return _finish(res.results), res


def kernel(pred, label):
    pred = np.asarray(pred, dtype=np.float32)
    label = np.asarray(label, dtype=np.float32)
    out, _ = _run(pred, label)
    return out


# revision 15
# speedup vs baseline: 1.5777x; 1.5777x over previous
"""Chamfer loss kernel for 8 Trainium2 NeuronCores.

Math: dist2[n, m] = ||pred_n||^2 + ||label_m||^2 - 2 pred_n . label_m
computed as a single K=16 matmul with augmented operands. Every operand
is split into an fp16 (hi, lo) pair (Dekker-style), so the fp16 matmul
(1 cycle/row on PE, vs 4 for fp32) reproduces fp32-level accuracy:
    cross terms: (ah+al).(ch+cl) -> 12 rows of pairwise products
    norm terms:  ||p||^2 and ||l||^2 as hi/lo pairs against ones -> 4 rows
Sharding: pred rows split across the 8 cores (1024 each); labels replicated.
Each core emits:
    rowmin [128, 8]  - min_m dist2 for its 1024 preds (partition p, block a)
    colmin [1, 8192] - min over its local preds for every label
Host: sqrt+mean of rowmins; cross-core min of colmins then sqrt+mean.
sqrt is monotonic so mins are taken on squared distances.
"""

import sys

for _p in ("/opt/trn_rl_repo", "/root/.axon_site/_ro/trn_rl_repo"):
    if _p not in sys.path:
        sys.path.append(_p)

import numpy as np

import concourse.bacc as bacc
import concourse.bass as bass
import concourse.mybir as mybir
import concourse.tile as tile
from concourse import bass_isa
from concourse.bass_utils import run_bass_kernel_spmd

F32 = mybir.dt.float32
F16 = mybir.dt.float16
KAUG = 16  # augmented contraction dim (fp16 hi/lo pairs)
SCALE = 1024  # 2^10: keeps scaled -dist2 out of fp16 subnormal range
# (overflow past fp16 max for far pairs is harmless: -inf loses the max)

N_CORES = 8
N = 8192  # preds (total)
M = 8192  # labels
NLOC = N // N_CORES  # preds per core
P = 128  # partitions
NBLK = NLOC // P  # pred blocks per core (8)
SUPER = 2048  # psum supertile width (4 banks)
MSUP = M // SUPER  # label supertiles (4)
MM = 512  # moving width per matmul

_nc_cache = None


def _build_nc():
    nc = bacc.Bacc(None, target_bir_lowering=False)

    predT_d = nc.dram_tensor("predT", [KAUG, NLOC], F16, kind="ExternalInput")
    labelT_d = nc.dram_tensor("labelT", [KAUG, M], F16, kind="ExternalInput")
    rowmin_d = nc.dram_tensor("rowmin", [P, NBLK], F16, kind="ExternalOutput")
    colmin_d = nc.dram_tensor("colmin", [1, M], F16, kind="ExternalOutput")

    AX = mybir.AxisListType
    OP = mybir.AluOpType

    with tile.TileContext(nc) as tc:
        with (
            tc.tile_pool(name="const", bufs=1) as cpool,
            tc.tile_pool(name="psum", bufs=2, space=bass.MemorySpace.PSUM) as ppool,
            tc.tile_pool(name="work", bufs=3) as wpool,
        ):
            predT_s = cpool.tile([KAUG, NLOC], F16)
            labelT_s = cpool.tile([KAUG, M], F16)
            nc.sync.dma_start(predT_s[:], predT_d[:])
            nc.sync.dma_start(labelT_s[:], labelT_d[:])

            # all mins are taken as max over SCALE * -dist2 in fp16
            colacc = cpool.tile([P, M], F16)
            rowfull = cpool.tile([P, NBLK, SUPER], F16)
            rowneg = cpool.tile([P, NBLK], F16)

            for a in range(NBLK):
                rowacc = rowfull[:, a, :]
                for b in range(MSUP):
                    ps = ppool.tile([P, SUPER], F32, tag="ps")
                    for k in range(SUPER // MM):
                        off = b * SUPER + k * MM
                        nc.tensor.matmul(
                            ps[:, k * MM : (k + 1) * MM],
                            predT_s[:, a * P : (a + 1) * P],
                            labelT_s[:, off : off + MM],
                            start=True,
                            stop=True,
                        )
                    # ACT drains PSUM -> negated, scaled fp16
                    if b == 0:
                        cp = rowacc  # row-side init lands directly here
                    else:
                        cp = wpool.tile([P, SUPER], F16, tag="cp")
                        nc.scalar.mul(cp[:], ps[:], -float(SCALE))
                        nc.vector.tensor_max(rowacc, rowacc, cp[:])
                    if b == 0:
                        nc.scalar.mul(cp, ps[:], -float(SCALE))
                    # col-side accumulate on DVE
                    dst = colacc[:, b * SUPER : (b + 1) * SUPER]
                    if a == 0:
                        nc.vector.tensor_copy(dst, cp if b == 0 else cp[:])
                    else:
                        nc.vector.tensor_max(dst, dst, cp if b == 0 else cp[:])

            # row epilogue over all 8 blocks at once via 3D APs:
            # fold 2048 -> 1024 -> 512, then reduce the innermost 512
            nc.vector.tensor_max(
                rowfull[:, :, 0:1024], rowfull[:, :, 0:1024], rowfull[:, :, 1024:2048]
            )
            nc.vector.tensor_max(
                rowfull[:, :, 0:512], rowfull[:, :, 0:512], rowfull[:, :, 512:1024]
            )
            nc.vector.tensor_reduce(
                rowneg[:], rowfull[:, :, 0:512], axis=AX.X, op=OP.max
            )

            # fold 128 partitions for the label-side mins: max of negated
            colred = cpool.tile([P, M], F16)
            nc.gpsimd.partition_all_reduce(
                colred[:], colacc[:], channels=P, reduce_op=bass_isa.ReduceOp.max
            )

            nc.sync.dma_start(rowmin_d[:], rowneg[:])
            nc.sync.dma_start(colmin_d[:], colred[0:1, :])

    nc.finalize()
    return nc


def _get_nc():
    global _nc_cache
    if _nc_cache is None:
        _nc_cache = _build_nc()
    return _nc_cache


def _make_inputs(pred, label):
    f16 = np.float16
    m2p = -2.0 * pred  # exact in fp32
    ah = m2p.astype(f16)
    al = (m2p - ah.astype(np.float32)).astype(f16)
    ch = label.astype(f16)
    cl = (label - ch.astype(np.float32)).astype(f16)
    pn = (pred.astype(np.float64) ** 2).sum(axis=1)
    ln = (label.astype(np.float64) ** 2).sum(axis=1)
    pnh = pn.astype(f16)
    pnl = (pn - pnh.astype(np.float64)).astype(f16)
    lnh = ln.astype(f16)
    lnl = (ln - lnh.astype(np.float64)).astype(f16)

    predT = np.empty((KAUG, N), f16)
    labelT = np.empty((KAUG, M), f16)
    predT[0:3] = ah.T
    predT[3:6] = ah.T
    predT[6:9] = al.T
    predT[9:12] = al.T
    predT[12] = pnh
    predT[13] = pnl
    predT[14] = 1.0
    predT[15] = 1.0
    labelT[0:3] = ch.T
    labelT[3:6] = cl.T
    labelT[6:9] = ch.T
    labelT[9:12] = cl.T
    labelT[12] = 1.0
    labelT[13] = 1.0
    labelT[14] = lnh
    labelT[15] = lnl
    return [
        {
            "predT": np.ascontiguousarray(predT[:, c * NLOC : (c + 1) * NLOC]),
            "labelT": labelT,
        }
        for c in range(N_CORES)
    ]


def _finish(results):
    inv = -1.0 / SCALE  # device outputs are SCALE * -dist2
    rowmins = inv * np.stack([r["rowmin"] for r in results]).astype(np.float64)
    colmins = inv * np.stack([r["colmin"][0] for r in results]).astype(np.float64)
    dis_xy = np.sqrt(np.maximum(rowmins, 0.0)).mean()
    colmin = colmins.min(axis=0)
    dis_yx = np.sqrt(np.maximum(colmin, 0.0)).mean()
    return np.float32(dis_xy + dis_yx)


def _run(pred, label, trace=False, **kw):
    nc = _get_nc()
    in_maps = _make_inputs(pred, label)
    res = run_bass_kernel_spmd(nc, in_maps, list(range(N_CORES)), trace=trace, **kw)
    return _finish(res.results), res


def kernel(pred, label):
    pred = np.asarray(pred, dtype=np.float32)
    label = np.asarray(label, dtype=np.float32)
    out, _ = _run(pred, label)
    return out


# revision 23
# speedup vs baseline: 1.6426x; 1.0412x over previous
"""Chamfer loss kernel for 8 Trainium2 NeuronCores.

Math: dist2[n, m] = ||pred_n||^2 + ||label_m||^2 - 2 pred_n . label_m
computed as a single K=16 matmul with augmented operands. Every operand
is split into an fp16 (hi, lo) pair (Dekker-style), so the fp16 matmul
(1 cycle/row on PE, vs 4 for fp32) reproduces fp32-level accuracy:
    cross terms: (ah+al).(ch+cl) -> 12 rows of pairwise products
    norm terms:  ||p||^2 and ||l||^2 as hi/lo pairs against ones -> 4 rows
Sharding: pred rows split across the 8 cores (1024 each); labels replicated.
Each core emits:
    rowmin [128, 8]  - min_m dist2 for its 1024 preds (partition p, block a)
    colmin [1, 8192] - min over its local preds for every label
Host: sqrt+mean of rowmins; cross-core min of colmins then sqrt+mean.
sqrt is monotonic so mins are taken on squared distances.
"""

import sys

for _p in ("/opt/trn_rl_repo", "/root/.axon_site/_ro/trn_rl_repo"):
    if _p not in sys.path:
        sys.path.append(_p)

import numpy as np

import concourse.bacc as bacc
import concourse.bass as bass
import concourse.mybir as mybir
import concourse.tile as tile
from concourse import bass_isa
from concourse.bass_utils import run_bass_kernel_spmd

F32 = mybir.dt.float32
F16 = mybir.dt.float16
KAUG = 16  # augmented contraction dim (fp16 hi/lo pairs)
SCALE = 256  # 2^8: lifts scaled -dist2 clear of fp16 subnormals while
# keeping the largest pair distances (~73 * 256) well under fp16 max

N_CORES = 8
N = 8192  # preds (total)
M = 8192  # labels
NLOC = N // N_CORES  # preds per core
P = 128  # partitions
NBLK = NLOC // P  # pred blocks per core (8)
SUPER = 2048  # psum supertile width (4 banks)
MSUP = M // SUPER  # label supertiles (4)
MM = 512  # moving width per matmul

_nc_cache = None


def _build_nc():
    nc = bacc.Bacc(None, target_bir_lowering=False)

    predT_d = nc.dram_tensor("predT", [KAUG, NLOC], F16, kind="ExternalInput")
    labelT_d = nc.dram_tensor("labelT", [KAUG, M], F16, kind="ExternalInput")
    ident_d = nc.dram_tensor("ident", [P, P], F16, kind="ExternalInput")
    rowmin_d = nc.dram_tensor("rowmin", [P, NBLK], F16, kind="ExternalOutput")
    colmin_d = nc.dram_tensor("colmin", [P, M // P], F16, kind="ExternalOutput")

    AX = mybir.AxisListType
    OP = mybir.AluOpType

    with tile.TileContext(nc) as tc:
        with (
            tc.tile_pool(name="const", bufs=1) as cpool,
            tc.tile_pool(name="psum", bufs=2, space=bass.MemorySpace.PSUM) as ppool,
            tc.tile_pool(name="work", bufs=2) as wpool,
        ):
            predT_s = cpool.tile([KAUG, NLOC], F16)
            labelT_s = cpool.tile([KAUG, M], F16)
            ident_s = cpool.tile([P, P], F16)
            nc.sync.dma_start(predT_s[:], predT_d[:])
            nc.sync.dma_start(labelT_s[:], labelT_d[:])
            nc.sync.dma_start(ident_s[:], ident_d[:])

            # all mins are taken as max over SCALE * -dist2 in fp16
            colacc = cpool.tile([P, M], F16)
            rowneg = cpool.tile([P, NBLK], F16)

            for a in range(NBLK):
                rowacc = None
                for b in range(MSUP):
                    ps = ppool.tile([P, SUPER], F32, tag="ps")
                    for k in range(SUPER // MM):
                        off = b * SUPER + k * MM
                        nc.tensor.matmul(
                            ps[:, k * MM : (k + 1) * MM],
                            predT_s[:, a * P : (a + 1) * P],
                            labelT_s[:, off : off + MM],
                            start=True,
                            stop=True,
                        )
                    # ACT drains PSUM -> negated, scaled fp16. The drain
                    # lands directly where one consumer wants it: in colacc
                    # for the first pred block (col init), in rowacc for
                    # b == 0 (row init); elsewhere a scratch tile.
                    dst = colacc[:, b * SUPER : (b + 1) * SUPER]
                    if a == 0:
                        cp = dst
                        nc.scalar.mul(cp, ps[:], -float(SCALE))
                        if b == 0:
                            rowacc = wpool.tile([P, SUPER], F16, tag="rowacc")
                            nc.vector.tensor_copy(rowacc[:], cp)
                    elif b == 0:
                        rowacc = wpool.tile([P, SUPER], F16, tag="rowacc")
                        cp = rowacc[:]
                        nc.scalar.mul(cp, ps[:], -float(SCALE))
                        nc.vector.tensor_max(dst, dst, cp)
                    else:
                        cpt = wpool.tile([P, SUPER], F16, tag=f"cp{b}")
                        cp = cpt[:]
                        nc.scalar.mul(cp, ps[:], -float(SCALE))
                        nc.vector.tensor_max(dst, dst, cp)
                    if b > 0:
                        nc.vector.tensor_max(rowacc[:], rowacc[:], cp)
                nc.vector.tensor_reduce(
                    rowneg[:, a : a + 1], rowacc[:], axis=AX.X, op=OP.max
                )

            # label-side partition fold: PE-transpose 128x128 chunks into
            # PSUM, then row-reduce the transposed chunks on DVE
            colneg = cpool.tile([P, M // P], F16)
            NT = 16  # chunks per transpose round (4 PSUM banks)
            for r in range(M // P // NT):
                pt = ppool.tile([P, NT, P], F16, tag="ps")
                for t in range(NT):
                    j = r * NT + t
                    nc.tensor.transpose(
                        pt[:, t, :], colacc[:, j * P : (j + 1) * P], ident_s[:]
                    )
                nc.vector.tensor_reduce(
                    colneg[:, r * NT : (r + 1) * NT], pt[:], axis=AX.X, op=OP.max
                )

            nc.sync.dma_start(rowmin_d[:], rowneg[:])
            nc.sync.dma_start(colmin_d[:], colneg[:])

    nc.finalize()
    return nc


def _get_nc():
    global _nc_cache
    if _nc_cache is None:
        _nc_cache = _build_nc()
    return _nc_cache


def _make_inputs(pred, label):
    f16 = np.float16
    m2p = -2.0 * pred  # exact in fp32
    ah = m2p.astype(f16)
    al = (m2p - ah.astype(np.float32)).astype(f16)
    ch = label.astype(f16)
    cl = (label - ch.astype(np.float32)).astype(f16)
    pn = (pred.astype(np.float64) ** 2).sum(axis=1)
    ln = (label.astype(np.float64) ** 2).sum(axis=1)
    pnh = pn.astype(f16)
    pnl = (pn - pnh.astype(np.float64)).astype(f16)
    lnh = ln.astype(f16)
    lnl = (ln - lnh.astype(np.float64)).astype(f16)

    predT = np.empty((KAUG, N), f16)
    labelT = np.empty((KAUG, M), f16)
    predT[0:3] = ah.T
    predT[3:6] = ah.T
    predT[6:9] = al.T
    predT[9:12] = al.T
    predT[12] = pnh
    predT[13] = pnl
    predT[14] = 1.0
    predT[15] = 1.0
    labelT[0:3] = ch.T
    labelT[3:6] = cl.T
    labelT[6:9] = ch.T
    labelT[9:12] = cl.T
    labelT[12] = 1.0
    labelT[13] = 1.0
    labelT[14] = lnh
    labelT[15] = lnl
    ident = np.eye(P, dtype=f16)
    return [
        {
            "predT": np.ascontiguousarray(predT[:, c * NLOC : (c + 1) * NLOC]),
            "labelT": labelT,
            "ident": ident,
        }
        for c in range(N_CORES)
    ]


def _finish(results):
    inv = -1.0 / SCALE  # device outputs are SCALE * -dist2
    rowmins = inv * np.stack([r["rowmin"] for r in results]).astype(np.float64)
    # colmin: [cores, 128, 64]; entry (p, j) is label m = j*128+p. Mean is
    # order-independent; only the cross-core min needs aligned (p, j).
    colnegs = np.stack([r["colmin"] for r in results]).astype(np.float64)
    colmin = inv * colnegs.max(axis=0)
    dis_xy = np.sqrt(np.maximum(rowmins, 0.0)).mean()
    dis_yx = np.sqrt(np.maximum(colmin, 0.0)).mean()
    return np.float32(dis_xy + dis_yx)


def _run(pred, label, trace=False, **kw):
    nc = _get_nc()
    in_maps = _make_inputs(pred, label)
    res = run_bass_kernel_spmd(nc, in_maps, list(range(N_CORES)), trace=trace, **kw)
    return _finish(res.results), res


def kernel(pred, label):
    pred = np.asarray(pred, dtype=np.float32)
    label = np.asarray(label, dtype=np.float32)
    out, _ = _run(pred, label)
    return out
